# revision 43
# baseline (speedup 1.0000x reference)
"""Trainium2 Bass kernel for grouped per-channel linears (nn_GroupedLinearsAdvanced).

Math: out[b, o, d] = sum_i x[b, i, d] * W[d, i, o] + bias[d, o]
with x: [16, 128, 4096] f32, W: [4096, 128, 128] f32, bias: [4096, 128] f32,
out: [16, 128, 4096] f32.

Sharding: channel dim D=4096 split into 8 contiguous slabs of 512 channels,
one per NeuronCore; x slices replicated per-slab, no cross-device reduction.

The kernel is HBM-DMA-bound: per core it must stream W (16.8 MB bf16)
+ x (2.1 MB) in and out (2.1 MB bf16) back at a measured per-core DMA
cap of ~330-360 GB/s, so everything is organized to keep the two HWDGE
rings (sync=SP, scalar=Activation) streaming back-to-back:

  - host pre-permutes/casts inputs to bf16 so every DMA moves long
    contiguous per-partition runs (W tiles: 16 KB/partition),
  - x slab resident in SBUF (layout [i, dl*16+b]), one chunk per ring
    at the head of each FIFO,
  - W streams through SBUF in ~2 MB tiles alternating between the two
    rings; all W buffers are distinct (no pool reuse), so the ring FIFOs
    never wait on compute,
  - per channel: one bf16 matmul  PS[o, b] = W_d.T @ x_d.T (lhsT = W_d);
    32 channels accumulate side-by-side into one 512-f32 PSUM bank,
  - DVE evacuates each bank into grouped bf16 SBUF staging tiles; the
    grouped out DMAs are issued *after* the whole W stream in each
    ring's FIFO, so they can never head-of-line-block a W tile, and by
    the time the rings drain the W stream the data is long evacuated,
  - the last W tiles are small (32 ch) so the compute that trails the
    final W bytes is short,
  - bias (when nonzero) seeds each PSUM bank via a bf16 one-hot
    expansion matmul; skipped entirely for the all-zero bias here.

Accuracy: bf16 inputs + f32 PSUM accumulate + bf16 output gives
rel err ~3.3e-3 on this problem (gate 2e-2). fp8 W was measured at
4.9e-2 — fails the gate, so 2-byte operands are the byte floor.

MM_DTYPE picks the tensor-engine path for x/W:
  "f32"  — exact fp32 (hardware runs 2 half-speed passes per matmul),
  "f32r" — same fp32 bytes, single-pass reduced-precision PE mode,
  "bf16" — bf16 in/out as above (default),
  "mix3" — bf16 hi+lo split, ~1e-5 accuracy at fp32 byte cost.
"""

import ml_dtypes
import numpy as np

from concourse import bacc, mybir, tile
from concourse.bass_utils import run_bass_kernel_spmd

B = 16           # batch
IN_D = 128       # contraction dim (SBUF partitions)
OUT_D = 128      # per-channel output dim
D_TOTAL = 4096   # channels
NCORES = 8
D_C = D_TOTAL // NCORES      # 512 channels per core
BANK_CH = 32                 # channels per PSUM bank (32*16 = 512 fp32 = 1 bank)
N_BANKS = D_C // BANK_CH     # 16

X_COLS = D_C * B                 # 8192
BN_COLS = N_BANKS * OUT_D        # 2048
EH_COLS = BANK_CH * B            # 512
CB_COLS = BN_COLS + EH_COLS      # bias + one-hot constant tensor

F32 = mybir.dt.float32
BF16 = mybir.dt.bfloat16

MM_DTYPE = "bf16"

_DT = {
    "f32": (F32, np.float32),
    "f32r": (mybir.dt.float32r, np.float32),
    # bf16: rel err ~2e-3 on this problem (gate is 2e-2); halves W/x HBM
    # traffic vs fp32/mix3 and needs one single-pass matmul per channel.
    # Output is also written bf16 (host upcasts) to halve out traffic.
    "bf16": (BF16, ml_dtypes.bfloat16),
    # mix3: W and x split into bf16 hi+lo parts; 3 single-pass matmuls
    # per channel (hi*hi + lo*hi + hi*lo) recover ~1e-5 accuracy while
    # keeping bf16 tensor-engine throughput. Same HBM bytes as fp32.
    "mix3": (BF16, ml_dtypes.bfloat16),
}

_OUT_DT = {"bf16": (BF16, ml_dtypes.bfloat16)}  # else f32

# (W tile channel sizes,
#  out-DMA bank groups (start_bank, n_banks, issue_after_tile, ring))
# issue_after_tile: W-tile index after whose DMA the group's out DMA is
# placed in that ring's FIFO (None = after the whole W stream). Groups
# are only scheduled where their last bank is evacuated well before the
# FIFO can reach them, so they never stall the stream.
_PLANS = {
    "big": ([128, 128, 128, 64, 32, 32],
            [(0, 4, None, 0), (4, 4, None, 1), (8, 4, None, 0),
             (12, 2, None, 1), (14, 1, None, 0), (15, 1, None, 1)]),
    "mid": ([64] * 7 + [32, 32],
            [(0, 4, None, 0), (4, 4, None, 1), (8, 4, None, 0),
             (12, 2, None, 1), (14, 1, None, 0), (15, 1, None, 1)]),
    "ilv": ([64] * 7 + [32, 32],
            [(0, 4, 3, 1), (4, 4, 4, 0), (8, 4, 7, 1),
             (12, 2, 8, 0), (14, 1, None, 1), (15, 1, None, 0)]),
    # 3 tail out DMAs: fewer ring handoff bubbles after the W stream.
    "tail3": ([64] * 7 + [32, 32],
              [(0, 8, None, 0), (8, 7, None, 1), (15, 1, None, 0)]),
    # Bulk out writes on the SWDGE (gpsimd) ring mid-stream; only the
    # last two banks' writes ride the HWDGE tails. Wins if HBM writes
    # don't contend with the HWDGE read stream.
    "swout": ([64] * 7 + [32, 32],
              [(0, 4, 2, 2), (4, 4, 4, 2), (8, 4, 6, 2),
               (12, 2, 8, 2), (14, 1, None, 0), (15, 1, None, 1)]),
    # Small head tiles + 4 x-chunks: engage both rings at full depth
    # faster to shorten the startup ramp.
    "ramp": ([32, 32] + [64] * 6 + [32, 32],
             [(0, 4, None, 0), (4, 4, None, 1), (8, 4, None, 0),
              (12, 2, None, 1), (14, 1, None, 0), (15, 1, None, 1)],
             4),
    # Byte-balanced rings (9 MB each incl. x) so both rings' W streams
    # finish together and the trailing compute chain starts ~1 us
    # earlier than with plain alternation (10 vs 8 MB).
    "bal": ([64] * 7 + [32, 32],
            [(0, 4, None, 0), (4, 4, None, 1), (8, 4, None, 0),
             (12, 2, None, 1), (14, 1, None, 0), (15, 1, None, 1)],
            2,
            [0, 1, 0, 1, 0, 1, 1, 0, 0]),
    # bal with out bytes also balanced 1.0/1.0 MB across the rings.
    "bal2": ([64] * 7 + [32, 32],
             [(0, 4, None, 0), (4, 4, None, 1), (8, 4, None, 0),
              (12, 2, None, 1), (14, 1, None, 1), (15, 1, None, 1)],
             2,
             [0, 1, 0, 1, 0, 1, 1, 0, 0]),
    # bal + the two tail W tiles each arrive as two half-DMAs so the
    # first 16 channels' matmuls overlap the second half's transfer,
    # shortening the post-stream compute chain.
    "bal3": ([64] * 7 + [32, 32],
             [(0, 4, None, 0), (4, 4, None, 1), (8, 4, None, 0),
              (12, 2, None, 1), (14, 1, None, 0), (15, 1, None, 1)],
             2,
             [0, 1, 0, 1, 0, 1, 1, 0, 0],
             2),
}
TILE_PLAN = _PLANS["bal"]

_cached = {}


def _build(mode, has_bias):
    dt_mm, _ = _DT[mode]
    nparts = 2 if mode == "mix3" else 1  # hi/lo operand copies
    nc = bacc.Bacc()
    xc = nc.dram_tensor("xc", [IN_D, nparts * X_COLS], dt_mm, kind="ExternalInput")
    wr = nc.dram_tensor(
        "wr", [IN_D, nparts * D_C * OUT_D], dt_mm, kind="ExternalInput"
    )
    if has_bias:
        cb = nc.dram_tensor("cb", [BANK_CH, CB_COLS], BF16, kind="ExternalInput")
    dt_out = _OUT_DT.get(mode, (F32, np.float32))[0]
    outr = nc.dram_tensor("outr", [OUT_D, D_C * B], dt_out, kind="ExternalOutput")

    # Channels per W tile. Few, large DMAs: each DMA instruction costs
    # ~0.9 us of completion-semaphore latency on its ring, so big head
    # tiles minimize instruction count while small tail tiles keep the
    # trailing compute (after the last W byte lands) short.
    if nparts == 1:
        tile_sizes = TILE_PLAN[0]
        out_groups = TILE_PLAN[1]
        n_xchunks = TILE_PLAN[2] if len(TILE_PLAN) > 2 else 2
        ring_assign = (
            TILE_PLAN[3]
            if len(TILE_PLAN) > 3
            else [t % 2 for t in range(len(tile_sizes))]
        )
        split_tail = TILE_PLAN[4] if len(TILE_PLAN) > 4 else 0
    else:
        n_xchunks = 2
        ring_assign = None
        split_tail = 0
        tile_ch0 = 64 // nparts
        tile_sizes = [tile_ch0] * (D_C // tile_ch0)
        out_groups = [(g, 1, None, g % 2) for g in range(N_BANKS)]
    assert sum(tile_sizes) == D_C
    group_of = {}
    for gi, (g0, ng, _, _) in enumerate(out_groups):
        for g in range(g0, g0 + ng):
            group_of[g] = (gi, g0, ng)
    wcols_per_ch = nparts * OUT_D
    with tile.TileContext(nc) as tc:
        with (
            tc.tile_pool(name="xp", bufs=1) as xp,
            tc.tile_pool(
                name="wp",
                bufs=sum(c == max(tile_sizes) for c in tile_sizes)
                if nparts == 1
                else 6,
            ) as wp,
            tc.tile_pool(name="wps", bufs=3) as wps,
            tc.tile_pool(name="op", bufs=len(out_groups)) as op,
            tc.tile_pool(name="pp", bufs=8, space="PSUM") as pp,
        ):
            XC = xp.tile([IN_D, nparts * X_COLS], dt_mm)
            # Chunks so early banks can start before the back half lands;
            # chunk-major order so bank 0 gets hi AND lo slices first.
            # One chunk per HWDGE ring, ahead of the W tiles in each FIFO.
            xch = X_COLS // n_xchunks
            for ch in range(n_xchunks):
                xeng = nc.sync if ch % 2 == 0 else nc.scalar
                for p in range(nparts):
                    lo = p * X_COLS + ch * xch
                    xeng.dma_start(
                        XC[:, lo:lo + xch], xc[:, lo:lo + xch]
                    )
            if has_bias:
                CB = xp.tile([BANK_CH, CB_COLS], BF16)
                nc.scalar.dma_start(CB[:], cb[:])

            rings = [nc.sync, nc.scalar, nc.gpsimd]

            def emit_out(g0, ng, ring):
                rings[ring].dma_start(
                    outr[:, g0 * BANK_CH * B:(g0 + ng) * BANK_CH * B],
                    ob_of[g0][:],
                )

            g = 0
            ch0 = 0
            ob_of = {}
            for t, tile_ch in enumerate(tile_sizes):
                banks_per_tile = tile_ch // BANK_CH
                wpool = wp if tile_ch == max(tile_sizes) else wps
                WT = wpool.tile([IN_D, tile_ch * wcols_per_ch], dt_mm)
                # Alternate the two HWDGE rings so W transfers overlap.
                # Each ring is a FIFO: an out DMA placed too early would
                # head-of-line-block later W tiles on compute of earlier
                # ones, so groups are placed per the plan's safety margin.
                weng = rings[ring_assign[t] if ring_assign else t % 2]
                lo = ch0 * wcols_per_ch
                hi = (ch0 + tile_ch) * wcols_per_ch
                if split_tail and t >= len(tile_sizes) - split_tail:
                    half = tile_ch * wcols_per_ch // 2
                    weng.dma_start(WT[:, :half], wr[:, lo:lo + half])
                    weng.dma_start(WT[:, half:], wr[:, lo + half:hi])
                else:
                    weng.dma_start(WT[:], wr[:, lo:hi])
                ch0 += tile_ch
                for g0, ng, t_after, ring in out_groups:
                    if t_after == t:
                        emit_out(g0, ng, ring)
                for h in range(banks_per_tile):
                    PS = pp.tile([OUT_D, BANK_CH * B], F32)
                    if has_bias:
                        # Seed bank with bias: PS[o, j*16+b] = bias[g*32+j, o].
                        nc.tensor.matmul(
                            PS[:],
                            CB[:, g * OUT_D:(g + 1) * OUT_D],
                            CB[:, BN_COLS:CB_COLS],
                            start=True,
                            stop=False,
                        )
                    for j in range(BANK_CH):
                        jt = h * BANK_CH + j
                        dl = g * BANK_CH + j
                        out_sl = PS[:, j * B:(j + 1) * B]
                        whi = WT[:, jt * wcols_per_ch:jt * wcols_per_ch + OUT_D]
                        xhi = XC[:, dl * B:(dl + 1) * B]
                        nc.tensor.matmul(
                            out_sl,
                            whi,
                            xhi,
                            start=(not has_bias) and j == 0,
                            stop=(mode != "mix3") and (j == BANK_CH - 1),
                        )
                        if mode == "mix3":
                            wlo = WT[
                                :,
                                jt * wcols_per_ch + OUT_D:(jt + 1) * wcols_per_ch,
                            ]
                            xlo = XC[:, X_COLS + dl * B:X_COLS + (dl + 1) * B]
                            nc.tensor.matmul(
                                out_sl, whi, xlo, start=False, stop=False
                            )
                            nc.tensor.matmul(
                                out_sl,
                                wlo,
                                xhi,
                                start=False,
                                stop=(j == BANK_CH - 1),
                            )
                    gi, g0, ng = group_of[g]
                    if g == g0:
                        OB = op.tile([OUT_D, ng * BANK_CH * B], dt_out)
                        ob_of[g0] = OB
                    off = (g - g0) * BANK_CH * B
                    nc.vector.tensor_copy(
                        ob_of[g0][:, off:off + BANK_CH * B], PS[:]
                    )
                    g += 1

            # Remaining out DMAs after the full W stream in each ring's
            # FIFO; their data is evacuated by the time the rings drain.
            for g0, ng, t_after, ring in out_groups:
                if t_after is None:
                    emit_out(g0, ng, ring)

    nc.finalize()
    return nc


def _pack_x(x, sl):
    # [b, i, dslab] -> [i, dl*16+b]
    return np.ascontiguousarray(x[:, :, sl].transpose(1, 2, 0)).reshape(
        IN_D, X_COLS
    )


def _pack_bias(b, sl, eh):
    bnr = np.ascontiguousarray(
        b[sl].reshape(N_BANKS, BANK_CH, OUT_D).transpose(1, 0, 2)
    ).reshape(BANK_CH, BN_COLS)
    cbv = np.zeros((BANK_CH, CB_COLS), dtype=ml_dtypes.bfloat16)
    cbv[:, :BN_COLS] = bnr.astype(ml_dtypes.bfloat16)
    cbv[:, BN_COLS:] = eh.astype(ml_dtypes.bfloat16)
    return cbv


def _prep_core_inputs(x, W, b, mode, has_bias):
    _, np_mm = _DT[mode]
    eh = np.repeat(np.eye(BANK_CH, dtype=np.float32), B, axis=1)
    if mode == "mix3":
        bf = ml_dtypes.bfloat16
        xh = x.astype(bf)
        xl = (x - xh.astype(np.float32)).astype(bf)
        Wh = W.astype(bf)
        Wl = (W - Wh.astype(np.float32)).astype(bf)
    in_maps = []
    for c in range(NCORES):
        sl = slice(c * D_C, (c + 1) * D_C)
        if mode == "mix3":
            xcv = np.concatenate(
                [_pack_x(xh.astype(np.float32), sl), _pack_x(xl.astype(np.float32), sl)],
                axis=1,
            ).astype(bf)
            wrv = np.ascontiguousarray(
                np.stack(
                    [Wh[sl].transpose(1, 0, 2), Wl[sl].transpose(1, 0, 2)],
                    axis=2,
                )
            ).reshape(IN_D, D_C * 2 * OUT_D)
            m = {"xc": xcv, "wr": wrv}
            if has_bias:
                m["cb"] = _pack_bias(b, sl, eh)
            in_maps.append(m)
            continue
        xr = _pack_x(x, sl).astype(np_mm, copy=False)
        wrv = (
            np.ascontiguousarray(W[sl].transpose(1, 0, 2))
            .reshape(IN_D, D_C * OUT_D)
            .astype(np_mm, copy=False)
        )
        m = {"xc": xr, "wr": wrv}
        if has_bias:
            m["cb"] = _pack_bias(b, sl, eh)
        in_maps.append(m)
    return in_maps


def run(inputs, trace=False, mode=None):
    mode = mode or MM_DTYPE
    x = np.asarray(inputs["x"], dtype=np.float32)
    W = np.asarray(inputs["W"], dtype=np.float32)
    b = np.asarray(inputs["b"], dtype=np.float32)
    has_bias = bool(np.any(b))
    key = (mode, has_bias)
    if key not in _cached:
        _cached[key] = _build(mode, has_bias)
    in_maps = _prep_core_inputs(x, W, b, mode, has_bias)
    res = run_bass_kernel_spmd(
        _cached[key], in_maps, core_ids=list(range(NCORES)), trace=trace
    )
    out = np.empty((B, OUT_D, D_TOTAL), dtype=np.float32)
    for c in range(NCORES):
        sl = slice(c * D_C, (c + 1) * D_C)
        out[:, :, sl] = (
            res.results[c]["outr"]
            .astype(np.float32, copy=False)
            .reshape(OUT_D, D_C, B)
            .transpose(2, 0, 1)
        )
    return out, res


def kernel(**inputs):
    out, _ = run(inputs)
    return out



# revision 46
# speedup vs baseline: 1.0137x; 1.0137x over previous
"""Trainium2 Bass kernel for grouped per-channel linears (nn_GroupedLinearsAdvanced).

Math: out[b, o, d] = sum_i x[b, i, d] * W[d, i, o] + bias[d, o]
with x: [16, 128, 4096] f32, W: [4096, 128, 128] f32, bias: [4096, 128] f32,
out: [16, 128, 4096] f32.

Sharding: channel dim D=4096 split into 8 contiguous slabs of 512 channels,
one per NeuronCore; x slices replicated per-slab, no cross-device reduction.

The kernel is HBM-DMA-bound: per core it must stream W (16.8 MB bf16)
+ x (2.1 MB) in and out (2.1 MB bf16) back at a measured per-core DMA
cap of ~330-360 GB/s, so everything is organized to keep the two HWDGE
rings (sync=SP, scalar=Activation) streaming back-to-back:

  - host pre-permutes/casts inputs to bf16 so every DMA moves long
    contiguous per-partition runs (W tiles: 16 KB/partition),
  - x slab resident in SBUF (layout [i, dl*16+b]), one chunk per ring
    at the head of each FIFO,
  - W streams through SBUF in ~2 MB tiles alternating between the two
    rings; all W buffers are distinct (no pool reuse), so the ring FIFOs
    never wait on compute,
  - per channel: one bf16 matmul  PS[o, b] = W_d.T @ x_d.T (lhsT = W_d);
    32 channels accumulate side-by-side into one 512-f32 PSUM bank,
  - DVE evacuates each bank into grouped bf16 SBUF staging tiles; the
    grouped out DMAs are issued *after* the whole W stream in each
    ring's FIFO, so they can never head-of-line-block a W tile, and by
    the time the rings drain the W stream the data is long evacuated,
  - the last W tiles are small (32 ch) so the compute that trails the
    final W bytes is short,
  - bias (when nonzero) seeds each PSUM bank via a bf16 one-hot
    expansion matmul; skipped entirely for the all-zero bias here.

Accuracy: bf16 inputs + f32 PSUM accumulate + bf16 output gives
rel err ~3.3e-3 on this problem (gate 2e-2). fp8 W was measured at
4.9e-2 — fails the gate, so 2-byte operands are the byte floor.

MM_DTYPE picks the tensor-engine path for x/W:
  "f32"  — exact fp32 (hardware runs 2 half-speed passes per matmul),
  "f32r" — same fp32 bytes, single-pass reduced-precision PE mode,
  "bf16" — bf16 in/out as above (default),
  "mix3" — bf16 hi+lo split, ~1e-5 accuracy at fp32 byte cost.
"""

import ml_dtypes
import numpy as np

from concourse import bacc, mybir, tile
from concourse.bass_utils import run_bass_kernel_spmd

B = 16           # batch
IN_D = 128       # contraction dim (SBUF partitions)
OUT_D = 128      # per-channel output dim
D_TOTAL = 4096   # channels
NCORES = 8
D_C = D_TOTAL // NCORES      # 512 channels per core
BANK_CH = 32                 # channels per PSUM bank (32*16 = 512 fp32 = 1 bank)
N_BANKS = D_C // BANK_CH     # 16

X_COLS = D_C * B                 # 8192
BN_COLS = N_BANKS * OUT_D        # 2048
EH_COLS = BANK_CH * B            # 512
CB_COLS = BN_COLS + EH_COLS      # bias + one-hot constant tensor

F32 = mybir.dt.float32
BF16 = mybir.dt.bfloat16

MM_DTYPE = "bf16"

_DT = {
    "f32": (F32, np.float32),
    "f32r": (mybir.dt.float32r, np.float32),
    # bf16: rel err ~2e-3 on this problem (gate is 2e-2); halves W/x HBM
    # traffic vs fp32/mix3 and needs one single-pass matmul per channel.
    # Output is also written bf16 (host upcasts) to halve out traffic.
    "bf16": (BF16, ml_dtypes.bfloat16),
    # mix3: W and x split into bf16 hi+lo parts; 3 single-pass matmuls
    # per channel (hi*hi + lo*hi + hi*lo) recover ~1e-5 accuracy while
    # keeping bf16 tensor-engine throughput. Same HBM bytes as fp32.
    "mix3": (BF16, ml_dtypes.bfloat16),
}

_OUT_DT = {"bf16": (BF16, ml_dtypes.bfloat16)}  # else f32

# (W tile channel sizes,
#  out-DMA bank groups (start_bank, n_banks, issue_after_tile, ring))
# issue_after_tile: W-tile index after whose DMA the group's out DMA is
# placed in that ring's FIFO (None = after the whole W stream). Groups
# are only scheduled where their last bank is evacuated well before the
# FIFO can reach them, so they never stall the stream.
_PLANS = {
    "big": ([128, 128, 128, 64, 32, 32],
            [(0, 4, None, 0), (4, 4, None, 1), (8, 4, None, 0),
             (12, 2, None, 1), (14, 1, None, 0), (15, 1, None, 1)]),
    "mid": ([64] * 7 + [32, 32],
            [(0, 4, None, 0), (4, 4, None, 1), (8, 4, None, 0),
             (12, 2, None, 1), (14, 1, None, 0), (15, 1, None, 1)]),
    "ilv": ([64] * 7 + [32, 32],
            [(0, 4, 3, 1), (4, 4, 4, 0), (8, 4, 7, 1),
             (12, 2, 8, 0), (14, 1, None, 1), (15, 1, None, 0)]),
    # 3 tail out DMAs: fewer ring handoff bubbles after the W stream.
    "tail3": ([64] * 7 + [32, 32],
              [(0, 8, None, 0), (8, 7, None, 1), (15, 1, None, 0)]),
    # Bulk out writes on the SWDGE (gpsimd) ring mid-stream; only the
    # last two banks' writes ride the HWDGE tails. Wins if HBM writes
    # don't contend with the HWDGE read stream.
    "swout": ([64] * 7 + [32, 32],
              [(0, 4, 2, 2), (4, 4, 4, 2), (8, 4, 6, 2),
               (12, 2, 8, 2), (14, 1, None, 0), (15, 1, None, 1)]),
    # Small head tiles + 4 x-chunks: engage both rings at full depth
    # faster to shorten the startup ramp.
    "ramp": ([32, 32] + [64] * 6 + [32, 32],
             [(0, 4, None, 0), (4, 4, None, 1), (8, 4, None, 0),
              (12, 2, None, 1), (14, 1, None, 0), (15, 1, None, 1)],
             4),
    # Byte-balanced rings (9 MB each incl. x) so both rings' W streams
    # finish together and the trailing compute chain starts ~1 us
    # earlier than with plain alternation (10 vs 8 MB).
    "bal": ([64] * 7 + [32, 32],
            [(0, 4, None, 0), (4, 4, None, 1), (8, 4, None, 0),
             (12, 2, None, 1), (14, 1, None, 0), (15, 1, None, 1)],
            2,
            [0, 1, 0, 1, 0, 1, 1, 0, 0]),
    # bal with out bytes also balanced 1.0/1.0 MB across the rings.
    "bal2": ([64] * 7 + [32, 32],
             [(0, 4, None, 0), (4, 4, None, 1), (8, 4, None, 0),
              (12, 2, None, 1), (14, 1, None, 1), (15, 1, None, 1)],
             2,
             [0, 1, 0, 1, 0, 1, 1, 0, 0]),
    # bal + the two tail W tiles each arrive as two half-DMAs so the
    # first 16 channels' matmuls overlap the second half's transfer,
    # shortening the post-stream compute chain.
    "bal3": ([64] * 7 + [32, 32],
             [(0, 4, None, 0), (4, 4, None, 1), (8, 4, None, 0),
              (12, 2, None, 1), (14, 1, None, 0), (15, 1, None, 1)],
             2,
             [0, 1, 0, 1, 0, 1, 1, 0, 0],
             2),
    # bal + last bank evacuated on the (idle) Activation engine so the
    # final two evacs run in parallel instead of queueing on the DVE.
    "bal4": ([64] * 7 + [32, 32],
             [(0, 4, None, 0), (4, 4, None, 1), (8, 4, None, 0),
              (12, 2, None, 1), (14, 1, None, 0), (15, 1, None, 1)],
             2,
             [0, 1, 0, 1, 0, 1, 1, 0, 0],
             0,
             True),
}
TILE_PLAN = _PLANS["bal"]

_cached = {}


def _build(mode, has_bias):
    dt_mm, _ = _DT[mode]
    nparts = 2 if mode == "mix3" else 1  # hi/lo operand copies
    nc = bacc.Bacc()
    xc = nc.dram_tensor("xc", [IN_D, nparts * X_COLS], dt_mm, kind="ExternalInput")
    wr = nc.dram_tensor(
        "wr", [IN_D, nparts * D_C * OUT_D], dt_mm, kind="ExternalInput"
    )
    if has_bias:
        cb = nc.dram_tensor("cb", [BANK_CH, CB_COLS], BF16, kind="ExternalInput")
    dt_out = _OUT_DT.get(mode, (F32, np.float32))[0]
    outr = nc.dram_tensor("outr", [OUT_D, D_C * B], dt_out, kind="ExternalOutput")

    # Channels per W tile. Few, large DMAs: each DMA instruction costs
    # ~0.9 us of completion-semaphore latency on its ring, so big head
    # tiles minimize instruction count while small tail tiles keep the
    # trailing compute (after the last W byte lands) short.
    if nparts == 1:
        tile_sizes = TILE_PLAN[0]
        out_groups = TILE_PLAN[1]
        n_xchunks = TILE_PLAN[2] if len(TILE_PLAN) > 2 else 2
        ring_assign = (
            TILE_PLAN[3]
            if len(TILE_PLAN) > 3
            else [t % 2 for t in range(len(tile_sizes))]
        )
        split_tail = TILE_PLAN[4] if len(TILE_PLAN) > 4 else 0
        act_last = TILE_PLAN[5] if len(TILE_PLAN) > 5 else False
    else:
        n_xchunks = 2
        ring_assign = None
        split_tail = 0
        act_last = False
        tile_ch0 = 64 // nparts
        tile_sizes = [tile_ch0] * (D_C // tile_ch0)
        out_groups = [(g, 1, None, g % 2) for g in range(N_BANKS)]
    assert sum(tile_sizes) == D_C
    group_of = {}
    for gi, (g0, ng, _, _) in enumerate(out_groups):
        for g in range(g0, g0 + ng):
            group_of[g] = (gi, g0, ng)
    wcols_per_ch = nparts * OUT_D
    with tile.TileContext(nc) as tc:
        with (
            tc.tile_pool(name="xp", bufs=1) as xp,
            tc.tile_pool(
                name="wp",
                bufs=sum(c == max(tile_sizes) for c in tile_sizes)
                if nparts == 1
                else 6,
            ) as wp,
            tc.tile_pool(name="wps", bufs=3) as wps,
            tc.tile_pool(name="op", bufs=len(out_groups)) as op,
            tc.tile_pool(name="pp", bufs=8, space="PSUM") as pp,
        ):
            XC = xp.tile([IN_D, nparts * X_COLS], dt_mm)
            # Chunks so early banks can start before the back half lands;
            # chunk-major order so bank 0 gets hi AND lo slices first.
            # One chunk per HWDGE ring, ahead of the W tiles in each FIFO.
            xch = X_COLS // n_xchunks
            for ch in range(n_xchunks):
                xeng = nc.sync if ch % 2 == 0 else nc.scalar
                for p in range(nparts):
                    lo = p * X_COLS + ch * xch
                    xeng.dma_start(
                        XC[:, lo:lo + xch], xc[:, lo:lo + xch]
                    )
            if has_bias:
                CB = xp.tile([BANK_CH, CB_COLS], BF16)
                nc.scalar.dma_start(CB[:], cb[:])

            rings = [nc.sync, nc.scalar, nc.gpsimd]

            def emit_out(g0, ng, ring):
                rings[ring].dma_start(
                    outr[:, g0 * BANK_CH * B:(g0 + ng) * BANK_CH * B],
                    ob_of[g0][:],
                )

            g = 0
            ch0 = 0
            ob_of = {}
            for t, tile_ch in enumerate(tile_sizes):
                banks_per_tile = tile_ch // BANK_CH
                wpool = wp if tile_ch == max(tile_sizes) else wps
                WT = wpool.tile([IN_D, tile_ch * wcols_per_ch], dt_mm)
                # Alternate the two HWDGE rings so W transfers overlap.
                # Each ring is a FIFO: an out DMA placed too early would
                # head-of-line-block later W tiles on compute of earlier
                # ones, so groups are placed per the plan's safety margin.
                weng = rings[ring_assign[t] if ring_assign else t % 2]
                lo = ch0 * wcols_per_ch
                hi = (ch0 + tile_ch) * wcols_per_ch
                if split_tail and t >= len(tile_sizes) - split_tail:
                    half = tile_ch * wcols_per_ch // 2
                    weng.dma_start(WT[:, :half], wr[:, lo:lo + half])
                    weng.dma_start(WT[:, half:], wr[:, lo + half:hi])
                else:
                    weng.dma_start(WT[:], wr[:, lo:hi])
                ch0 += tile_ch
                for g0, ng, t_after, ring in out_groups:
                    if t_after == t:
                        emit_out(g0, ng, ring)
                for h in range(banks_per_tile):
                    PS = pp.tile([OUT_D, BANK_CH * B], F32)
                    if has_bias:
                        # Seed bank with bias: PS[o, j*16+b] = bias[g*32+j, o].
                        nc.tensor.matmul(
                            PS[:],
                            CB[:, g * OUT_D:(g + 1) * OUT_D],
                            CB[:, BN_COLS:CB_COLS],
                            start=True,
                            stop=False,
                        )
                    for j in range(BANK_CH):
                        jt = h * BANK_CH + j
                        dl = g * BANK_CH + j
                        out_sl = PS[:, j * B:(j + 1) * B]
                        whi = WT[:, jt * wcols_per_ch:jt * wcols_per_ch + OUT_D]
                        xhi = XC[:, dl * B:(dl + 1) * B]
                        nc.tensor.matmul(
                            out_sl,
                            whi,
                            xhi,
                            start=(not has_bias) and j == 0,
                            stop=(mode != "mix3") and (j == BANK_CH - 1),
                        )
                        if mode == "mix3":
                            wlo = WT[
                                :,
                                jt * wcols_per_ch + OUT_D:(jt + 1) * wcols_per_ch,
                            ]
                            xlo = XC[:, X_COLS + dl * B:X_COLS + (dl + 1) * B]
                            nc.tensor.matmul(
                                out_sl, whi, xlo, start=False, stop=False
                            )
                            nc.tensor.matmul(
                                out_sl,
                                wlo,
                                xhi,
                                start=False,
                                stop=(j == BANK_CH - 1),
                            )
                    gi, g0, ng = group_of[g]
                    if g == g0:
                        OB = op.tile([OUT_D, ng * BANK_CH * B], dt_out)
                        ob_of[g0] = OB
                    off = (g - g0) * BANK_CH * B
                    dst = ob_of[g0][:, off:off + BANK_CH * B]
                    if act_last and g == N_BANKS - 1:
                        nc.scalar.copy(dst, PS[:])
                    else:
                        nc.vector.tensor_copy(dst, PS[:])
                    g += 1

            # Remaining out DMAs after the full W stream in each ring's
            # FIFO; their data is evacuated by the time the rings drain.
            for g0, ng, t_after, ring in out_groups:
                if t_after is None:
                    emit_out(g0, ng, ring)

    nc.finalize()
    return nc


def _pack_x(x, sl):
    # [b, i, dslab] -> [i, dl*16+b]
    return np.ascontiguousarray(x[:, :, sl].transpose(1, 2, 0)).reshape(
        IN_D, X_COLS
    )


def _pack_bias(b, sl, eh):
    bnr = np.ascontiguousarray(
        b[sl].reshape(N_BANKS, BANK_CH, OUT_D).transpose(1, 0, 2)
    ).reshape(BANK_CH, BN_COLS)
    cbv = np.zeros((BANK_CH, CB_COLS), dtype=ml_dtypes.bfloat16)
    cbv[:, :BN_COLS] = bnr.astype(ml_dtypes.bfloat16)
    cbv[:, BN_COLS:] = eh.astype(ml_dtypes.bfloat16)
    return cbv


def _prep_core_inputs(x, W, b, mode, has_bias):
    _, np_mm = _DT[mode]
    eh = np.repeat(np.eye(BANK_CH, dtype=np.float32), B, axis=1)
    if mode == "mix3":
        bf = ml_dtypes.bfloat16
        xh = x.astype(bf)
        xl = (x - xh.astype(np.float32)).astype(bf)
        Wh = W.astype(bf)
        Wl = (W - Wh.astype(np.float32)).astype(bf)
    in_maps = []
    for c in range(NCORES):
        sl = slice(c * D_C, (c + 1) * D_C)
        if mode == "mix3":
            xcv = np.concatenate(
                [_pack_x(xh.astype(np.float32), sl), _pack_x(xl.astype(np.float32), sl)],
                axis=1,
            ).astype(bf)
            wrv = np.ascontiguousarray(
                np.stack(
                    [Wh[sl].transpose(1, 0, 2), Wl[sl].transpose(1, 0, 2)],
                    axis=2,
                )
            ).reshape(IN_D, D_C * 2 * OUT_D)
            m = {"xc": xcv, "wr": wrv}
            if has_bias:
                m["cb"] = _pack_bias(b, sl, eh)
            in_maps.append(m)
            continue
        xr = _pack_x(x, sl).astype(np_mm, copy=False)
        wrv = (
            np.ascontiguousarray(W[sl].transpose(1, 0, 2))
            .reshape(IN_D, D_C * OUT_D)
            .astype(np_mm, copy=False)
        )
        m = {"xc": xr, "wr": wrv}
        if has_bias:
            m["cb"] = _pack_bias(b, sl, eh)
        in_maps.append(m)
    return in_maps


def run(inputs, trace=False, mode=None):
    mode = mode or MM_DTYPE
    x = np.asarray(inputs["x"], dtype=np.float32)
    W = np.asarray(inputs["W"], dtype=np.float32)
    b = np.asarray(inputs["b"], dtype=np.float32)
    has_bias = bool(np.any(b))
    key = (mode, has_bias)
    if key not in _cached:
        _cached[key] = _build(mode, has_bias)
    in_maps = _prep_core_inputs(x, W, b, mode, has_bias)
    res = run_bass_kernel_spmd(
        _cached[key], in_maps, core_ids=list(range(NCORES)), trace=trace
    )
    out = np.empty((B, OUT_D, D_TOTAL), dtype=np.float32)
    for c in range(NCORES):
        sl = slice(c * D_C, (c + 1) * D_C)
        out[:, :, sl] = (
            res.results[c]["outr"]
            .astype(np.float32, copy=False)
            .reshape(OUT_D, D_C, B)
            .transpose(2, 0, 1)
        )
    return out, res


def kernel(**inputs):
    out, _ = run(inputs)
    return out



# revision 48
# speedup vs baseline: 1.0166x; 1.0028x over previous
"""Trainium2 Bass kernel for grouped per-channel linears (nn_GroupedLinearsAdvanced).

Math: out[b, o, d] = sum_i x[b, i, d] * W[d, i, o] + bias[d, o]
with x: [16, 128, 4096] f32, W: [4096, 128, 128] f32, bias: [4096, 128] f32,
out: [16, 128, 4096] f32.

Sharding: channel dim D=4096 split into 8 contiguous slabs of 512 channels,
one per NeuronCore; x slices replicated per-slab, no cross-device reduction.

The kernel is HBM-DMA-bound: per core it must stream W (16.8 MB bf16)
+ x (2.1 MB) in and out (2.1 MB bf16) back at a measured per-core DMA
cap of ~330-360 GB/s, so everything is organized to keep the two HWDGE
rings (sync=SP, scalar=Activation) streaming back-to-back:

  - host pre-permutes/casts inputs to bf16 so every DMA moves long
    contiguous per-partition runs (W tiles: 16 KB/partition),
  - x slab resident in SBUF (layout [i, dl*16+b]), one chunk per ring
    at the head of each FIFO,
  - W streams through SBUF in ~2 MB tiles alternating between the two
    rings; all W buffers are distinct (no pool reuse), so the ring FIFOs
    never wait on compute,
  - per channel: one bf16 matmul  PS[o, b] = W_d.T @ x_d.T (lhsT = W_d);
    32 channels accumulate side-by-side into one 512-f32 PSUM bank,
  - DVE evacuates each bank into grouped bf16 SBUF staging tiles; the
    grouped out DMAs are issued *after* the whole W stream in each
    ring's FIFO, so they can never head-of-line-block a W tile, and by
    the time the rings drain the W stream the data is long evacuated,
  - the last W tiles are small (32 ch) so the compute that trails the
    final W bytes is short,
  - bias (when nonzero) seeds each PSUM bank via a bf16 one-hot
    expansion matmul; skipped entirely for the all-zero bias here.

Accuracy: bf16 inputs + f32 PSUM accumulate + bf16 output gives
rel err ~3.3e-3 on this problem (gate 2e-2). fp8 W was measured at
4.9e-2 — fails the gate, so 2-byte operands are the byte floor.

MM_DTYPE picks the tensor-engine path for x/W:
  "f32"  — exact fp32 (hardware runs 2 half-speed passes per matmul),
  "f32r" — same fp32 bytes, single-pass reduced-precision PE mode,
  "bf16" — bf16 in/out as above (default),
  "mix3" — bf16 hi+lo split, ~1e-5 accuracy at fp32 byte cost.
"""

import ml_dtypes
import numpy as np

from concourse import bacc, mybir, tile
from concourse.bass_utils import run_bass_kernel_spmd

B = 16           # batch
IN_D = 128       # contraction dim (SBUF partitions)
OUT_D = 128      # per-channel output dim
D_TOTAL = 4096   # channels
NCORES = 8
D_C = D_TOTAL // NCORES      # 512 channels per core
BANK_CH = 32                 # channels per PSUM bank (32*16 = 512 fp32 = 1 bank)
N_BANKS = D_C // BANK_CH     # 16

X_COLS = D_C * B                 # 8192
BN_COLS = N_BANKS * OUT_D        # 2048
EH_COLS = BANK_CH * B            # 512
CB_COLS = BN_COLS + EH_COLS      # bias + one-hot constant tensor

F32 = mybir.dt.float32
BF16 = mybir.dt.bfloat16

MM_DTYPE = "bf16"

_DT = {
    "f32": (F32, np.float32),
    "f32r": (mybir.dt.float32r, np.float32),
    # bf16: rel err ~2e-3 on this problem (gate is 2e-2); halves W/x HBM
    # traffic vs fp32/mix3 and needs one single-pass matmul per channel.
    # Output is also written bf16 (host upcasts) to halve out traffic.
    "bf16": (BF16, ml_dtypes.bfloat16),
    # mix3: W and x split into bf16 hi+lo parts; 3 single-pass matmuls
    # per channel (hi*hi + lo*hi + hi*lo) recover ~1e-5 accuracy while
    # keeping bf16 tensor-engine throughput. Same HBM bytes as fp32.
    "mix3": (BF16, ml_dtypes.bfloat16),
}

_OUT_DT = {"bf16": (BF16, ml_dtypes.bfloat16)}  # else f32

# (W tile channel sizes,
#  out-DMA bank groups (start_bank, n_banks, issue_after_tile, ring))
# issue_after_tile: W-tile index after whose DMA the group's out DMA is
# placed in that ring's FIFO (None = after the whole W stream). Groups
# are only scheduled where their last bank is evacuated well before the
# FIFO can reach them, so they never stall the stream.
_PLANS = {
    "big": ([128, 128, 128, 64, 32, 32],
            [(0, 4, None, 0), (4, 4, None, 1), (8, 4, None, 0),
             (12, 2, None, 1), (14, 1, None, 0), (15, 1, None, 1)]),
    "mid": ([64] * 7 + [32, 32],
            [(0, 4, None, 0), (4, 4, None, 1), (8, 4, None, 0),
             (12, 2, None, 1), (14, 1, None, 0), (15, 1, None, 1)]),
    "ilv": ([64] * 7 + [32, 32],
            [(0, 4, 3, 1), (4, 4, 4, 0), (8, 4, 7, 1),
             (12, 2, 8, 0), (14, 1, None, 1), (15, 1, None, 0)]),
    # 3 tail out DMAs: fewer ring handoff bubbles after the W stream.
    "tail3": ([64] * 7 + [32, 32],
              [(0, 8, None, 0), (8, 7, None, 1), (15, 1, None, 0)]),
    # Bulk out writes on the SWDGE (gpsimd) ring mid-stream; only the
    # last two banks' writes ride the HWDGE tails. Wins if HBM writes
    # don't contend with the HWDGE read stream.
    "swout": ([64] * 7 + [32, 32],
              [(0, 4, 2, 2), (4, 4, 4, 2), (8, 4, 6, 2),
               (12, 2, 8, 2), (14, 1, None, 0), (15, 1, None, 1)]),
    # Small head tiles + 4 x-chunks: engage both rings at full depth
    # faster to shorten the startup ramp.
    "ramp": ([32, 32] + [64] * 6 + [32, 32],
             [(0, 4, None, 0), (4, 4, None, 1), (8, 4, None, 0),
              (12, 2, None, 1), (14, 1, None, 0), (15, 1, None, 1)],
             4),
    # Byte-balanced rings (9 MB each incl. x) so both rings' W streams
    # finish together and the trailing compute chain starts ~1 us
    # earlier than with plain alternation (10 vs 8 MB).
    "bal": ([64] * 7 + [32, 32],
            [(0, 4, None, 0), (4, 4, None, 1), (8, 4, None, 0),
             (12, 2, None, 1), (14, 1, None, 0), (15, 1, None, 1)],
            2,
            [0, 1, 0, 1, 0, 1, 1, 0, 0]),
    # bal with out bytes also balanced 1.0/1.0 MB across the rings.
    "bal2": ([64] * 7 + [32, 32],
             [(0, 4, None, 0), (4, 4, None, 1), (8, 4, None, 0),
              (12, 2, None, 1), (14, 1, None, 1), (15, 1, None, 1)],
             2,
             [0, 1, 0, 1, 0, 1, 1, 0, 0]),
    # bal + the two tail W tiles each arrive as two half-DMAs so the
    # first 16 channels' matmuls overlap the second half's transfer,
    # shortening the post-stream compute chain.
    "bal3": ([64] * 7 + [32, 32],
             [(0, 4, None, 0), (4, 4, None, 1), (8, 4, None, 0),
              (12, 2, None, 1), (14, 1, None, 0), (15, 1, None, 1)],
             2,
             [0, 1, 0, 1, 0, 1, 1, 0, 0],
             2),
    # bal + last bank evacuated on the (idle) Activation engine so the
    # final two evacs run in parallel instead of queueing on the DVE.
    "bal4": ([64] * 7 + [32, 32],
             [(0, 4, None, 0), (4, 4, None, 1), (8, 4, None, 0),
              (12, 2, None, 1), (14, 1, None, 0), (15, 1, None, 1)],
             2,
             [0, 1, 0, 1, 0, 1, 1, 0, 0],
             0,
             True),
    # Large head tiles (fewer W instructions) with balanced 8/8 MB
    # rings and the proven small-tail structure.
    "bigbal": ([128, 128, 64, 64, 64, 32, 32],
               [(0, 4, None, 0), (4, 4, None, 1), (8, 4, None, 0),
                (12, 2, None, 1), (14, 1, None, 0), (15, 1, None, 1)],
               2,
               [0, 1, 0, 1, 1, 0, 0]),
}
TILE_PLAN = _PLANS["bigbal"]

_cached = {}


def _build(mode, has_bias):
    dt_mm, _ = _DT[mode]
    nparts = 2 if mode == "mix3" else 1  # hi/lo operand copies
    nc = bacc.Bacc()
    xc = nc.dram_tensor("xc", [IN_D, nparts * X_COLS], dt_mm, kind="ExternalInput")
    wr = nc.dram_tensor(
        "wr", [IN_D, nparts * D_C * OUT_D], dt_mm, kind="ExternalInput"
    )
    if has_bias:
        cb = nc.dram_tensor("cb", [BANK_CH, CB_COLS], BF16, kind="ExternalInput")
    dt_out = _OUT_DT.get(mode, (F32, np.float32))[0]
    outr = nc.dram_tensor("outr", [OUT_D, D_C * B], dt_out, kind="ExternalOutput")

    # Channels per W tile. Few, large DMAs: each DMA instruction costs
    # ~0.9 us of completion-semaphore latency on its ring, so big head
    # tiles minimize instruction count while small tail tiles keep the
    # trailing compute (after the last W byte lands) short.
    if nparts == 1:
        tile_sizes = TILE_PLAN[0]
        out_groups = TILE_PLAN[1]
        n_xchunks = TILE_PLAN[2] if len(TILE_PLAN) > 2 else 2
        ring_assign = (
            TILE_PLAN[3]
            if len(TILE_PLAN) > 3
            else [t % 2 for t in range(len(tile_sizes))]
        )
        split_tail = TILE_PLAN[4] if len(TILE_PLAN) > 4 else 0
        act_last = TILE_PLAN[5] if len(TILE_PLAN) > 5 else False
    else:
        n_xchunks = 2
        ring_assign = None
        split_tail = 0
        act_last = False
        tile_ch0 = 64 // nparts
        tile_sizes = [tile_ch0] * (D_C // tile_ch0)
        out_groups = [(g, 1, None, g % 2) for g in range(N_BANKS)]
    assert sum(tile_sizes) == D_C
    group_of = {}
    for gi, (g0, ng, _, _) in enumerate(out_groups):
        for g in range(g0, g0 + ng):
            group_of[g] = (gi, g0, ng)
    wcols_per_ch = nparts * OUT_D
    with tile.TileContext(nc) as tc:
        with (
            tc.tile_pool(name="xp", bufs=1) as xp,
            tc.tile_pool(
                name="wp",
                bufs=sum(c == max(tile_sizes) for c in tile_sizes)
                if nparts == 1
                else 6,
            ) as wp,
            tc.tile_pool(name="wps", bufs=3) as wps,
            tc.tile_pool(name="op", bufs=len(out_groups)) as op,
            tc.tile_pool(name="pp", bufs=8, space="PSUM") as pp,
        ):
            XC = xp.tile([IN_D, nparts * X_COLS], dt_mm)
            # Chunks so early banks can start before the back half lands;
            # chunk-major order so bank 0 gets hi AND lo slices first.
            # One chunk per HWDGE ring, ahead of the W tiles in each FIFO.
            xch = X_COLS // n_xchunks
            for ch in range(n_xchunks):
                xeng = nc.sync if ch % 2 == 0 else nc.scalar
                for p in range(nparts):
                    lo = p * X_COLS + ch * xch
                    xeng.dma_start(
                        XC[:, lo:lo + xch], xc[:, lo:lo + xch]
                    )
            if has_bias:
                CB = xp.tile([BANK_CH, CB_COLS], BF16)
                nc.scalar.dma_start(CB[:], cb[:])

            rings = [nc.sync, nc.scalar, nc.gpsimd]

            def emit_out(g0, ng, ring):
                rings[ring].dma_start(
                    outr[:, g0 * BANK_CH * B:(g0 + ng) * BANK_CH * B],
                    ob_of[g0][:],
                )

            g = 0
            ch0 = 0
            ob_of = {}
            for t, tile_ch in enumerate(tile_sizes):
                banks_per_tile = tile_ch // BANK_CH
                wpool = wp if tile_ch == max(tile_sizes) else wps
                WT = wpool.tile([IN_D, tile_ch * wcols_per_ch], dt_mm)
                # Alternate the two HWDGE rings so W transfers overlap.
                # Each ring is a FIFO: an out DMA placed too early would
                # head-of-line-block later W tiles on compute of earlier
                # ones, so groups are placed per the plan's safety margin.
                weng = rings[ring_assign[t] if ring_assign else t % 2]
                lo = ch0 * wcols_per_ch
                hi = (ch0 + tile_ch) * wcols_per_ch
                if split_tail and t >= len(tile_sizes) - split_tail:
                    half = tile_ch * wcols_per_ch // 2
                    weng.dma_start(WT[:, :half], wr[:, lo:lo + half])
                    weng.dma_start(WT[:, half:], wr[:, lo + half:hi])
                else:
                    weng.dma_start(WT[:], wr[:, lo:hi])
                ch0 += tile_ch
                for g0, ng, t_after, ring in out_groups:
                    if t_after == t:
                        emit_out(g0, ng, ring)
                for h in range(banks_per_tile):
                    PS = pp.tile([OUT_D, BANK_CH * B], F32)
                    if has_bias:
                        # Seed bank with bias: PS[o, j*16+b] = bias[g*32+j, o].
                        nc.tensor.matmul(
                            PS[:],
                            CB[:, g * OUT_D:(g + 1) * OUT_D],
                            CB[:, BN_COLS:CB_COLS],
                            start=True,
                            stop=False,
                        )
                    for j in range(BANK_CH):
                        jt = h * BANK_CH + j
                        dl = g * BANK_CH + j
                        out_sl = PS[:, j * B:(j + 1) * B]
                        whi = WT[:, jt * wcols_per_ch:jt * wcols_per_ch + OUT_D]
                        xhi = XC[:, dl * B:(dl + 1) * B]
                        nc.tensor.matmul(
                            out_sl,
                            whi,
                            xhi,
                            start=(not has_bias) and j == 0,
                            stop=(mode != "mix3") and (j == BANK_CH - 1),
                        )
                        if mode == "mix3":
                            wlo = WT[
                                :,
                                jt * wcols_per_ch + OUT_D:(jt + 1) * wcols_per_ch,
                            ]
                            xlo = XC[:, X_COLS + dl * B:X_COLS + (dl + 1) * B]
                            nc.tensor.matmul(
                                out_sl, whi, xlo, start=False, stop=False
                            )
                            nc.tensor.matmul(
                                out_sl,
                                wlo,
                                xhi,
                                start=False,
                                stop=(j == BANK_CH - 1),
                            )
                    gi, g0, ng = group_of[g]
                    if g == g0:
                        OB = op.tile([OUT_D, ng * BANK_CH * B], dt_out)
                        ob_of[g0] = OB
                    off = (g - g0) * BANK_CH * B
                    dst = ob_of[g0][:, off:off + BANK_CH * B]
                    if act_last and g == N_BANKS - 1:
                        nc.scalar.copy(dst, PS[:])
                    else:
                        nc.vector.tensor_copy(dst, PS[:])
                    g += 1

            # Remaining out DMAs after the full W stream in each ring's
            # FIFO; their data is evacuated by the time the rings drain.
            for g0, ng, t_after, ring in out_groups:
                if t_after is None:
                    emit_out(g0, ng, ring)

    nc.finalize()
    return nc


def _pack_x(x, sl):
    # [b, i, dslab] -> [i, dl*16+b]
    return np.ascontiguousarray(x[:, :, sl].transpose(1, 2, 0)).reshape(
        IN_D, X_COLS
    )


def _pack_bias(b, sl, eh):
    bnr = np.ascontiguousarray(
        b[sl].reshape(N_BANKS, BANK_CH, OUT_D).transpose(1, 0, 2)
    ).reshape(BANK_CH, BN_COLS)
    cbv = np.zeros((BANK_CH, CB_COLS), dtype=ml_dtypes.bfloat16)
    cbv[:, :BN_COLS] = bnr.astype(ml_dtypes.bfloat16)
    cbv[:, BN_COLS:] = eh.astype(ml_dtypes.bfloat16)
    return cbv


def _prep_core_inputs(x, W, b, mode, has_bias):
    _, np_mm = _DT[mode]
    eh = np.repeat(np.eye(BANK_CH, dtype=np.float32), B, axis=1)
    if mode == "mix3":
        bf = ml_dtypes.bfloat16
        xh = x.astype(bf)
        xl = (x - xh.astype(np.float32)).astype(bf)
        Wh = W.astype(bf)
        Wl = (W - Wh.astype(np.float32)).astype(bf)
    in_maps = []
    for c in range(NCORES):
        sl = slice(c * D_C, (c + 1) * D_C)
        if mode == "mix3":
            xcv = np.concatenate(
                [_pack_x(xh.astype(np.float32), sl), _pack_x(xl.astype(np.float32), sl)],
                axis=1,
            ).astype(bf)
            wrv = np.ascontiguousarray(
                np.stack(
                    [Wh[sl].transpose(1, 0, 2), Wl[sl].transpose(1, 0, 2)],
                    axis=2,
                )
            ).reshape(IN_D, D_C * 2 * OUT_D)
            m = {"xc": xcv, "wr": wrv}
            if has_bias:
                m["cb"] = _pack_bias(b, sl, eh)
            in_maps.append(m)
            continue
        xr = _pack_x(x, sl).astype(np_mm, copy=False)
        wrv = (
            np.ascontiguousarray(W[sl].transpose(1, 0, 2))
            .reshape(IN_D, D_C * OUT_D)
            .astype(np_mm, copy=False)
        )
        m = {"xc": xr, "wr": wrv}
        if has_bias:
            m["cb"] = _pack_bias(b, sl, eh)
        in_maps.append(m)
    return in_maps


def run(inputs, trace=False, mode=None):
    mode = mode or MM_DTYPE
    x = np.asarray(inputs["x"], dtype=np.float32)
    W = np.asarray(inputs["W"], dtype=np.float32)
    b = np.asarray(inputs["b"], dtype=np.float32)
    has_bias = bool(np.any(b))
    key = (mode, has_bias)
    if key not in _cached:
        _cached[key] = _build(mode, has_bias)
    in_maps = _prep_core_inputs(x, W, b, mode, has_bias)
    res = run_bass_kernel_spmd(
        _cached[key], in_maps, core_ids=list(range(NCORES)), trace=trace
    )
    out = np.empty((B, OUT_D, D_TOTAL), dtype=np.float32)
    for c in range(NCORES):
        sl = slice(c * D_C, (c + 1) * D_C)
        out[:, :, sl] = (
            res.results[c]["outr"]
            .astype(np.float32, copy=False)
            .reshape(OUT_D, D_C, B)
            .transpose(2, 0, 1)
        )
    return out, res


def kernel(**inputs):
    out, _ = run(inputs)
    return out

